# revision 6
# baseline (speedup 1.0000x reference)
"""Multi-head causal attention (b=2, s=2048, d=1024, 16 heads) on 8 NeuronCores.

Sharding: head-tensor-parallel attention + Ulysses-style AllToAll.
  - Core c (c=0..7) owns batch c//4 and heads [4*(c%4), 4*(c%4)+4).
  - Each core: x^T via PE transpose, q^T/k^T (head-dim-major) + v (natural)
    projections, causal flash-style attention for its 4 heads -> ctx^T.
  - Global 8-core AllToAll redistributes ctx^T so core j ends with all 1024
    ctx features for seq rows [256*j, 256*(j+1)) of BOTH batches.
  - Sequence-parallel out-projection (full Wo, bias via K=1 ones matmul).
  - Host reassembles the 8 [512, 1024] row-slices. No all-reduce needed.

Matmuls run in float32r (tf32-like, 1 cycle/row at N>=256) with fp32 PSUM
accumulation; softmax runs unnormalized-exp (logits ~ N(0,1), no overflow)
with the denominator produced by an appended ones-column in the AV matmul.
"""

import sys

if "/opt/trn_rl_repo" not in sys.path:
    sys.path.insert(0, "/opt/trn_rl_repo")

import numpy as np

import concourse.bass as bass
import concourse.mybir as mybir
import concourse.bacc as bacc
import concourse.tile as tile
from concourse import bass_utils
from concourse.masks import make_identity

dt = mybir.dt
AF = mybir.ActivationFunctionType
F32 = dt.float32
F32R = dt.float32r

S = 2048          # sequence length
D = 1024          # model dim
NH = 16           # total heads
DH = 64           # head dim
NC = 8            # cores
HPC = NH // (NC // 2)   # heads per core = 4
HW = HPC * DH     # per-core head width = 256
QC = 512          # query chunk (free dim of S^T / AV matmuls)
NQC = S // QC     # 4
NST = S // 128    # 16 seq tiles
NKD = D // 128    # 8 contraction chunks over D
VW = HPC * (DH + 1)     # v tile width per seq-tile: 4 heads x (64 v + 1 ones)
ROWS = S // NC    # 256 rows per core per batch after A2A

_CACHE = {}


def _build():
    nc = bacc.Bacc("TRN2", debug=False, num_devices=NC)
    xin = nc.dram_tensor("xin", [S, D], F32R, kind="ExternalInput").ap()
    wq = nc.dram_tensor("wq", [D, HW], F32R, kind="ExternalInput").ap()
    wk = nc.dram_tensor("wk", [D, HW], F32R, kind="ExternalInput").ap()
    wv = nc.dram_tensor("wv", [D, HW], F32R, kind="ExternalInput").ap()
    wo = nc.dram_tensor("wo", [D, D], F32R, kind="ExternalInput").ap()
    bo = nc.dram_tensor("bo", [1, D], F32R, kind="ExternalInput").ap()
    out_slice = nc.dram_tensor(
        "out_slice", [2 * ROWS, D], F32, kind="ExternalOutput"
    ).ap()

    with tile.TileContext(nc) as tc:
        with (
            tc.tile_pool(name="constp", bufs=1) as constp,
            tc.tile_pool(name="wpool", bufs=1) as wpool,
            tc.tile_pool(name="dramp", bufs=1, space="DRAM") as dramp,
            tc.tile_pool(name="psum", bufs=1, space="PSUM") as psum,
        ):
            ident_f32 = constp.tile([128, 128], F32)
            make_identity(nc, ident_f32)
            ident = constp.tile([128, 128], F32R)
            nc.vector.tensor_copy(ident[:], ident_f32[:])
            ones_f32 = constp.tile([128, 128], F32)
            nc.vector.memset(ones_f32[:], 1.0)
            ones = constp.tile([1, 128], F32R)
            nc.vector.tensor_copy(ones[:], ones_f32[0:1, :])
            bo_sb = constp.tile([1, D], F32R)
            nc.sync.dma_start(bo_sb[:], bo[:])

            # weights: chunk kd of W* lives at columns [kd*w, (kd+1)*w)
            wq_sb = wpool.tile([128, NKD * HW], F32R)
            wk_sb = wpool.tile([128, NKD * HW], F32R)
            wv_sb = wpool.tile([128, NKD * HW], F32R)
            wo_sb = wpool.tile([128, NKD * D], F32R)
            for kd in range(NKD):
                r = slice(kd * 128, (kd + 1) * 128)
                nc.sync.dma_start(wq_sb[:, kd * HW:(kd + 1) * HW], wq[r, :])
                nc.sync.dma_start(wk_sb[:, kd * HW:(kd + 1) * HW], wk[r, :])
                nc.sync.dma_start(wv_sb[:, kd * HW:(kd + 1) * HW], wv[r, :])
                nc.sync.dma_start(wo_sb[:, kd * D:(kd + 1) * D], wo[r, :])

            a2a_in = dramp.tile([NC, HW, ROWS], F32R)
            a2a_out = dramp.tile([NC, HW, ROWS], F32R)

            with tc.tile_pool(name="qkvp", bufs=1) as qkvp:
                qT = qkvp.tile([128, 2 * S], F32R)   # head pair hp at cols hp*S
                kT = qkvp.tile([128, 2 * S], F32R)
                v_sb = qkvp.tile([128, NST * VW], F32R)
                # ones columns interleaved in v tiles: col st*VW + h*(DH+1) + DH
                v_view = v_sb[:].rearrange(
                    "p (st h c) -> p st h c", st=NST, h=HPC, c=DH + 1
                )
                nc.vector.tensor_copy(
                    v_view[:, :, :, DH:DH + 1],
                    ones_f32[:, 0:NST * HPC].rearrange(
                        "p (st h o) -> p st h o", st=NST, h=HPC, o=1))

                with tc.tile_pool(name="xtp", bufs=1) as xtp:
                    xT = xtp.tile([128, NKD * S], F32R)  # chunk kd at cols kd*S
                    xT_view = xT[:].rearrange("p (kd s) -> p kd s", kd=NKD)

                    with tc.tile_pool(name="xstage", bufs=3) as xsp:
                        for st in range(NST):
                            xst = xsp.tile([128, D], F32R, name=f"xst{st}",
                                           tag="xst")
                            nc.sync.dma_start(
                                xst[:], xin[st * 128:(st + 1) * 128, :])
                            for g in range(2):
                                pt = psum.tile([128, 512], F32R,
                                               name=f"tp{st}_{g}", tag="tp")
                                for kq in range(4):
                                    kd = g * 4 + kq
                                    nc.tensor.transpose(
                                        pt[:, kq * 128:(kq + 1) * 128],
                                        xst[:, kd * 128:(kd + 1) * 128],
                                        ident[:],
                                    )
                                dest = xT_view[
                                    :, g * 4:(g + 1) * 4,
                                    st * 128:(st + 1) * 128]
                                src = pt[:].rearrange("p (kq s) -> p kq s", kq=4)
                                nc.vector.tensor_copy(dest, src)

                    # q^T / k^T projections, head-pair packed (M=128)
                    for hp in range(2):
                        for qc in range(NQC):
                            pq = psum.tile([128, QC], F32,
                                           name=f"pq{hp}_{qc}", tag="pj")
                            pk = psum.tile([128, QC], F32,
                                           name=f"pk{hp}_{qc}", tag="pj")
                            for kd in range(NKD):
                                wcol = slice(kd * HW + hp * 128,
                                             kd * HW + (hp + 1) * 128)
                                xchunk = xT[:, kd * S + qc * QC:
                                            kd * S + (qc + 1) * QC]
                                nc.tensor.matmul(
                                    pq[:], wq_sb[:, wcol], xchunk,
                                    start=(kd == 0), stop=(kd == NKD - 1))
                                nc.tensor.matmul(
                                    pk[:], wk_sb[:, wcol], xchunk,
                                    start=(kd == 0), stop=(kd == NKD - 1))
                            dst = slice(hp * S + qc * QC, hp * S + (qc + 1) * QC)
                            nc.scalar.copy(qT[:, dst], pq[:])
                            nc.scalar.copy(kT[:, dst], pk[:])

                    # v natural [s, dh], 4 heads packed with ones columns
                    for st in range(NST):
                        pv = psum.tile([128, HW], F32, name=f"pv{st}", tag="pj")
                        for kd in range(NKD):
                            nc.tensor.matmul(
                                pv[:],
                                xT_view[:, kd, st * 128:(st + 1) * 128],
                                wv_sb[:, kd * HW:(kd + 1) * HW],
                                start=(kd == 0), stop=(kd == NKD - 1))
                        nc.vector.tensor_copy(
                            v_view[:, st, :, 0:DH],
                            pv[:].rearrange("p (h d) -> p h d", h=HPC))

                # attention per (head, q-chunk)
                with (
                    tc.tile_pool(name="pep", bufs=3) as pep,
                    tc.tile_pool(name="nrm", bufs=2) as nrm,
                ):
                    for h in range(HPC):
                        hp, hl = h // 2, h % 2
                        prow = slice(hl * 64, (hl + 1) * 64)
                        for qc in range(NQC):
                            ctxp = psum.tile([DH + 1, QC], F32,
                                             name=f"ctx{h}_{qc}", tag="ctx")
                            nkt = 4 * (qc + 1)
                            for kt in range(nkt):
                                sp = psum.tile([128, QC], F32,
                                               name=f"sp{h}_{qc}_{kt}",
                                               tag="sp")
                                nc.tensor.matmul(
                                    sp[:],
                                    kT[prow, hp * S + kt * 128:
                                       hp * S + (kt + 1) * 128],
                                    qT[prow, hp * S + qc * QC:
                                       hp * S + (qc + 1) * QC],
                                    start=True, stop=True)
                                pe = pep.tile([128, QC], F32R,
                                              name=f"pe{h}_{qc}_{kt}",
                                              tag="pe")
                                nc.scalar.activation(
                                    pe[:], sp[:], AF.Exp, scale=0.125)
                                if kt >= 4 * qc:
                                    nc.gpsimd.affine_select(
                                        out=pe[:], in_=pe[:],
                                        compare_op=mybir.AluOpType.is_ge,
                                        fill=0.0,
                                        base=qc * QC - kt * 128,
                                        pattern=[[1, QC]],
                                        channel_multiplier=-1)
                                nc.tensor.matmul(
                                    ctxp[:],
                                    v_sb[:, kt * VW + h * (DH + 1):
                                         kt * VW + (h + 1) * (DH + 1)],
                                    pe[:],
                                    start=(kt == 0), stop=(kt == nkt - 1))
                            # normalize: ctx[d, q] /= denom[q] (row DH of ctxp)
                            rec = nrm.tile([1, QC], F32R,
                                           name=f"rec{h}_{qc}", tag="rec")
                            with nc.allow_low_precision(reason="f32r=fp32 bits"):
                                nc.vector.reciprocal(
                                    rec[:], ctxp[DH:DH + 1, :])
                            bc = psum.tile([DH, QC], F32,
                                           name=f"bc{h}_{qc}", tag="tp")
                            nc.tensor.matmul(bc[:], ones[:, 0:DH], rec[:],
                                             start=True, stop=True)
                            bcs = nrm.tile([DH, QC], F32R,
                                           name=f"bcs{h}_{qc}", tag="bcs")
                            nc.vector.tensor_copy(bcs[:], bc[:])
                            ctxn = nrm.tile([DH, QC], F32R,
                                            name=f"ctxn{h}_{qc}", tag="ctxn")
                            nc.vector.tensor_mul(
                                ctxn[:], ctxp[0:DH, :], bcs[:])
                            for half in range(2):
                                nc.sync.dma_start(
                                    a2a_in[2 * qc + half,
                                           h * DH:(h + 1) * DH, :],
                                    ctxn[:, half * ROWS:(half + 1) * ROWS])

                nc.gpsimd.collective_compute(
                    "AllToAll", mybir.AluOpType.bypass,
                    replica_groups=[list(range(NC))],
                    ins=[a2a_in.opt()], outs=[a2a_out.opt()])

            # out-projection, sequence-parallel: 2*ROWS rows, full D columns
            with (
                tc.tile_pool(name="opool", bufs=1) as opool,
                tc.tile_pool(name="ostage", bufs=4) as ostage,
            ):
                for bh in range(2):
                    ctxo = opool.tile([128, NKD * ROWS], F32R,
                                      name=f"ctxo{bh}", tag=f"ctxo{bh}")
                    for c in range(NKD):
                        blk = bh * 4 + c // 2
                        pr = slice((c % 2) * 128, (c % 2) * 128 + 128)
                        nc.sync.dma_start(
                            ctxo[:, c * ROWS:(c + 1) * ROWS],
                            a2a_out[blk, pr, :])
                    for ssub in range(ROWS // 128):
                        for nh in range(2):
                            po = psum.tile([128, 512], F32,
                                           name=f"po{bh}_{ssub}_{nh}",
                                           tag="pj")
                            for c in range(NKD):
                                nc.tensor.matmul(
                                    po[:],
                                    ctxo[:, c * ROWS + ssub * 128:
                                         c * ROWS + ssub * 128 + 128],
                                    wo_sb[:, c * D + nh * 512:
                                          c * D + (nh + 1) * 512],
                                    start=(c == 0), stop=False)
                            nc.tensor.matmul(
                                po[:], ones[:],
                                bo_sb[:, nh * 512:(nh + 1) * 512],
                                start=False, stop=True)
                            outt = ostage.tile([128, 512], F32,
                                               name=f"ot{bh}_{ssub}_{nh}",
                                               tag="outt")
                            nc.vector.tensor_copy(outt[:], po[:])
                            nc.sync.dma_start(
                                out_slice[bh * ROWS + ssub * 128:
                                          bh * ROWS + ssub * 128 + 128,
                                          nh * 512:(nh + 1) * 512],
                                outt[:])

    nc.compile()
    return nc


def kernel(x, Wq, Wk, Wv, Wo, bo):
    if "nc" not in _CACHE:
        _CACHE["nc"] = _build()
    nc = _CACHE["nc"]

    x = np.ascontiguousarray(np.asarray(x, dtype=np.float32))
    Wq = np.asarray(Wq, dtype=np.float32)
    Wk = np.asarray(Wk, dtype=np.float32)
    Wv = np.asarray(Wv, dtype=np.float32)
    Wo = np.ascontiguousarray(np.asarray(Wo, dtype=np.float32))
    bo = np.asarray(bo, dtype=np.float32).reshape(1, D)

    in_maps = []
    for c in range(NC):
        b, hg = c // 4, c % 4
        cols = slice(hg * HW, (hg + 1) * HW)
        in_maps.append({
            "xin": np.ascontiguousarray(x[b]),
            "wq": np.ascontiguousarray(Wq[:, cols]),
            "wk": np.ascontiguousarray(Wk[:, cols]),
            "wv": np.ascontiguousarray(Wv[:, cols]),
            "wo": Wo,
            "bo": bo,
        })

    res = bass_utils.run_bass_kernel_spmd(nc, in_maps, core_ids=list(range(NC)))

    out = np.empty((2, S, D), dtype=np.float32)
    for j in range(NC):
        sl = res.results[j]["out_slice"]
        out[0, j * ROWS:(j + 1) * ROWS] = sl[:ROWS]
        out[1, j * ROWS:(j + 1) * ROWS] = sl[ROWS:]
    return out


# revision 8
# speedup vs baseline: 1.1968x; 1.1968x over previous
"""Multi-head causal attention (b=2, s=2048, d=1024, 16 heads) on 8 NeuronCores.

Sharding: head-tensor-parallel attention + Ulysses-style AllToAll.
  - Core c (c=0..7) owns batch c//4 and heads [4*(c%4), 4*(c%4)+4).
  - Each core: x^T via PE transpose, q^T/k^T (head-dim-major) + v (natural)
    projections, causal flash-style attention for its 4 heads -> ctx^T.
  - Two global 8-core AllToAlls (one per head pair, so the first overlaps
    with attention compute of the second pair) redistribute ctx^T so core j
    ends with all 1024 ctx features for rows [256*j, 256*(j+1)) of BOTH
    batches.
  - Sequence-parallel out-projection (full Wo, bias via K=1 ones matmul).
  - Host reassembles the 8 [512, 1024] row-slices. No all-reduce needed.

Matmuls run in float32r (tf32-like, 1 cycle/row at N>=256) with fp32 PSUM
accumulation; softmax runs unnormalized-exp (logits ~ N(0,1), no overflow)
with the denominator produced by an appended ones-column in the AV matmul.
Causal masking multiplies exp tiles by 4 precomputed diagonal masks on the
vector engine (keeps GpSimd out of the inner loop; PE stays HAM-warm).
"""

import sys

if "/opt/trn_rl_repo" not in sys.path:
    sys.path.insert(0, "/opt/trn_rl_repo")

import numpy as np

import concourse.bass as bass
import concourse.mybir as mybir
import concourse.bacc as bacc
import concourse.tile as tile
from concourse import bass_utils
from concourse.masks import make_identity

dt = mybir.dt
AF = mybir.ActivationFunctionType
F32 = dt.float32
F32R = dt.float32r

S = 2048          # sequence length
D = 1024          # model dim
NH = 16           # total heads
DH = 64           # head dim
NC = 8            # cores
HPC = NH // (NC // 2)   # heads per core = 4
HW = HPC * DH     # per-core head width = 256
QC = 512          # query chunk (free dim of S^T / AV matmuls)
NQC = S // QC     # 4
NST = S // 128    # 16 seq tiles
NKD = D // 128    # 8 contraction chunks over D
VW = HPC * (DH + 1)     # v tile width per seq-tile: 4 heads x (64 v + 1 ones)
ROWS = S // NC    # 256 rows per core per batch after A2A

_CACHE = {}


def _build():
    nc = bacc.Bacc("TRN2", debug=False, num_devices=NC)
    xin = nc.dram_tensor("xin", [S, D], F32R, kind="ExternalInput").ap()
    wq = nc.dram_tensor("wq", [D, HW], F32R, kind="ExternalInput").ap()
    wk = nc.dram_tensor("wk", [D, HW], F32R, kind="ExternalInput").ap()
    wv = nc.dram_tensor("wv", [D, HW], F32R, kind="ExternalInput").ap()
    wo = nc.dram_tensor("wo", [D, D], F32R, kind="ExternalInput").ap()
    bo = nc.dram_tensor("bo", [1, D], F32R, kind="ExternalInput").ap()
    out_slice = nc.dram_tensor(
        "out_slice", [2 * ROWS, D], F32, kind="ExternalOutput"
    ).ap()

    with tile.TileContext(nc) as tc:
        with (
            tc.tile_pool(name="constp", bufs=1) as constp,
            tc.tile_pool(name="wpool", bufs=1) as wpool,
            tc.tile_pool(name="dramp", bufs=1, space="DRAM") as dramp,
            tc.tile_pool(name="psum", bufs=1, space="PSUM") as psum,
        ):
            ident_f32 = constp.tile([128, 128], F32)
            make_identity(nc, ident_f32)
            ident = constp.tile([128, 128], F32R)
            nc.vector.tensor_copy(ident[:], ident_f32[:])
            ones_f32 = constp.tile([128, 128], F32)
            nc.vector.memset(ones_f32[:], 1.0)
            ones = constp.tile([1, 128], F32R)
            nc.vector.tensor_copy(ones[:], ones_f32[0:1, :])
            bo_sb = constp.tile([1, D], F32R)
            nc.sync.dma_start(bo_sb[:], bo[:])

            # 4 diagonal causal masks: mask[j][k, q] = (q - k - 128*j >= 0)
            masks = []
            mask_f32 = constp.tile([128, QC], F32)
            for j in range(4):
                nc.vector.memset(mask_f32[:], 1.0)
                nc.gpsimd.affine_select(
                    out=mask_f32[:], in_=mask_f32[:],
                    compare_op=mybir.AluOpType.is_ge,
                    fill=0.0, base=-128 * j,
                    pattern=[[1, QC]], channel_multiplier=-1)
                m = constp.tile([128, QC], F32R, name=f"mask{j}", tag=f"mask{j}")
                nc.vector.tensor_copy(m[:], mask_f32[:])
                masks.append(m)

            # weights: chunk kd of W* lives at columns [kd*w, (kd+1)*w)
            wq_sb = wpool.tile([128, NKD * HW], F32R)
            wk_sb = wpool.tile([128, NKD * HW], F32R)
            wv_sb = wpool.tile([128, NKD * HW], F32R)
            wo_sb = wpool.tile([128, NKD * D], F32R)
            for kd in range(NKD):
                r = slice(kd * 128, (kd + 1) * 128)
                nc.sync.dma_start(wq_sb[:, kd * HW:(kd + 1) * HW], wq[r, :])
                nc.sync.dma_start(wk_sb[:, kd * HW:(kd + 1) * HW], wk[r, :])
                nc.sync.dma_start(wv_sb[:, kd * HW:(kd + 1) * HW], wv[r, :])
                nc.sync.dma_start(wo_sb[:, kd * D:(kd + 1) * D], wo[r, :])

            # A2A buffers, one pair per head-pair (hp) so hp0's collective
            # overlaps hp1's attention compute.
            a2a_in = [dramp.tile([NC, 128, ROWS], F32R, name=f"a2ai{i}",
                                 tag=f"a2ai{i}") for i in range(2)]
            a2a_out = [dramp.tile([NC, 128, ROWS], F32R, name=f"a2ao{i}",
                                  tag=f"a2ao{i}") for i in range(2)]

            with tc.tile_pool(name="qkvp", bufs=1) as qkvp:
                qT = qkvp.tile([128, 2 * S], F32R)   # head pair hp at cols hp*S
                kT = qkvp.tile([128, 2 * S], F32R)
                v_sb = qkvp.tile([128, NST * VW], F32R)
                # ones columns interleaved in v tiles: col st*VW + h*(DH+1) + DH
                v_view = v_sb[:].rearrange(
                    "p (st h c) -> p st h c", st=NST, h=HPC, c=DH + 1
                )
                nc.vector.tensor_copy(
                    v_view[:, :, :, DH:DH + 1],
                    ones_f32[:, 0:NST * HPC].rearrange(
                        "p (st h o) -> p st h o", st=NST, h=HPC, o=1))

                with tc.tile_pool(name="xtp", bufs=1) as xtp:
                    xT = xtp.tile([128, NKD * S], F32R)  # chunk kd at cols kd*S
                    xT_view = xT[:].rearrange("p (kd s) -> p kd s", kd=NKD)

                    with tc.tile_pool(name="xstage", bufs=3) as xsp:
                        for st in range(NST):
                            xst = xsp.tile([128, D], F32R, name=f"xst{st}",
                                           tag="xst")
                            nc.sync.dma_start(
                                xst[:], xin[st * 128:(st + 1) * 128, :])
                            for g in range(2):
                                pt = psum.tile([128, 512], F32R,
                                               name=f"tp{st}_{g}", tag="tp")
                                for kq in range(4):
                                    kd = g * 4 + kq
                                    nc.tensor.transpose(
                                        pt[:, kq * 128:(kq + 1) * 128],
                                        xst[:, kd * 128:(kd + 1) * 128],
                                        ident[:],
                                    )
                                dest = xT_view[
                                    :, g * 4:(g + 1) * 4,
                                    st * 128:(st + 1) * 128]
                                src = pt[:].rearrange("p (kq s) -> p kq s", kq=4)
                                nc.vector.tensor_copy(dest, src)

                    # q^T / k^T projections, head-pair packed (M=128)
                    for hp in range(2):
                        for qc in range(NQC):
                            pq = psum.tile([128, QC], F32,
                                           name=f"pq{hp}_{qc}", tag="pjq")
                            pk = psum.tile([128, QC], F32,
                                           name=f"pk{hp}_{qc}", tag="pjk")
                            for kd in range(NKD):
                                wcol = slice(kd * HW + hp * 128,
                                             kd * HW + (hp + 1) * 128)
                                xchunk = xT[:, kd * S + qc * QC:
                                            kd * S + (qc + 1) * QC]
                                nc.tensor.matmul(
                                    pq[:], wq_sb[:, wcol], xchunk,
                                    start=(kd == 0), stop=(kd == NKD - 1))
                                nc.tensor.matmul(
                                    pk[:], wk_sb[:, wcol], xchunk,
                                    start=(kd == 0), stop=(kd == NKD - 1))
                            dst = slice(hp * S + qc * QC, hp * S + (qc + 1) * QC)
                            nc.scalar.copy(qT[:, dst], pq[:])
                            nc.scalar.copy(kT[:, dst], pk[:])

                    # v natural [s, dh], 4 heads packed with ones columns
                    for st in range(NST):
                        pv = psum.tile([128, HW], F32, name=f"pv{st}",
                                       tag="pjq")
                        for kd in range(NKD):
                            nc.tensor.matmul(
                                pv[:],
                                xT_view[:, kd, st * 128:(st + 1) * 128],
                                wv_sb[:, kd * HW:(kd + 1) * HW],
                                start=(kd == 0), stop=(kd == NKD - 1))
                        nc.vector.tensor_copy(
                            v_view[:, st, :, 0:DH],
                            pv[:].rearrange("p (h d) -> p h d", h=HPC))

                # attention per (head, q-chunk), normalize software-pipelined
                # one iteration late so no engine queue stalls across iters
                with (
                    tc.tile_pool(name="pep", bufs=4) as pep,
                    tc.tile_pool(name="nrm", bufs=2) as nrm,
                ):
                    def normalize(h, qc, ctxp):
                        # ctx[d, q] /= denom[q] (denom = row DH of ctxp)
                        hp, hl = h // 2, h % 2
                        den = nrm.tile([1, QC], F32R,
                                       name=f"den{h}_{qc}", tag="den")
                        nc.scalar.copy(den[:], ctxp[DH:DH + 1, :])
                        bc = psum.tile([DH, QC], F32,
                                       name=f"bc{h}_{qc}", tag="tp")
                        nc.tensor.matmul(bc[:], ones[:, 0:DH], den[:],
                                         start=True, stop=True)
                        bcs = nrm.tile([DH, QC], F32,
                                       name=f"bcs{h}_{qc}", tag="bcs")
                        nc.vector.reciprocal_approx_fast(bcs[:], bc[:])
                        ctxn = nrm.tile([DH, QC], F32R,
                                        name=f"ctxn{h}_{qc}", tag="ctxn")
                        nc.vector.tensor_mul(ctxn[:], ctxp[0:DH, :], bcs[:])
                        for half in range(2):
                            nc.sync.dma_start(
                                a2a_in[hp][2 * qc + half,
                                           hl * DH:(hl + 1) * DH, :],
                                ctxn[:, half * ROWS:(half + 1) * ROWS])

                    pending = None  # (h, qc, ctxp) awaiting normalize
                    fire_cc = None  # head-pair whose collective is due
                    for h in range(HPC):
                        hp, hl = h // 2, h % 2
                        prow = slice(hl * 64, (hl + 1) * 64)
                        for qc in range(NQC):
                            ctxp = psum.tile([DH + 1, QC], F32,
                                             name=f"ctx{h}_{qc}", tag="ctx")
                            nkt = 4 * (qc + 1)
                            for kt in range(nkt):
                                sp = psum.tile([128, QC], F32,
                                               name=f"sp{h}_{qc}_{kt}",
                                               tag="sp")
                                nc.tensor.matmul(
                                    sp[:],
                                    kT[prow, hp * S + kt * 128:
                                       hp * S + (kt + 1) * 128],
                                    qT[prow, hp * S + qc * QC:
                                       hp * S + (qc + 1) * QC],
                                    start=True, stop=True)
                                pe = pep.tile([128, QC], F32R,
                                              name=f"pe{h}_{qc}_{kt}",
                                              tag="pe")
                                nc.scalar.activation(
                                    pe[:], sp[:], AF.Exp, scale=0.125)
                                if kt >= 4 * qc:
                                    nc.vector.tensor_mul(
                                        pe[:], pe[:], masks[kt - 4 * qc][:])
                                nc.tensor.matmul(
                                    ctxp[:],
                                    v_sb[:, kt * VW + h * (DH + 1):
                                         kt * VW + (h + 1) * (DH + 1)],
                                    pe[:],
                                    start=(kt == 0), stop=(kt == nkt - 1))
                                if kt == 1 and pending is not None:
                                    normalize(*pending)
                                    pending = None
                                    if fire_cc is not None:
                                        nc.gpsimd.collective_compute(
                                            "AllToAll",
                                            mybir.AluOpType.bypass,
                                            replica_groups=[list(range(NC))],
                                            ins=[a2a_in[fire_cc].opt()],
                                            outs=[a2a_out[fire_cc].opt()])
                                        fire_cc = None
                            pending = (h, qc, ctxp)
                        if h % 2 == 1:
                            fire_cc = hp
                    normalize(*pending)
                    nc.gpsimd.collective_compute(
                        "AllToAll", mybir.AluOpType.bypass,
                        replica_groups=[list(range(NC))],
                        ins=[a2a_in[1].opt()],
                        outs=[a2a_out[1].opt()])

            # out-projection, sequence-parallel: 2*ROWS rows, full D columns
            with (
                tc.tile_pool(name="opool", bufs=1) as opool,
                tc.tile_pool(name="ostage", bufs=4) as ostage,
            ):
                for bh in range(2):
                    ctxo = opool.tile([128, NKD * ROWS], F32R,
                                      name=f"ctxo{bh}", tag=f"ctxo{bh}")
                    for c in range(NKD):
                        nc.sync.dma_start(
                            ctxo[:, c * ROWS:(c + 1) * ROWS],
                            a2a_out[c % 2][bh * 4 + c // 2, :, :])
                    for ssub in range(ROWS // 128):
                        for nh in range(2):
                            po = psum.tile([128, 512], F32,
                                           name=f"po{bh}_{ssub}_{nh}",
                                           tag="pjk")
                            for c in range(NKD):
                                nc.tensor.matmul(
                                    po[:],
                                    ctxo[:, c * ROWS + ssub * 128:
                                         c * ROWS + ssub * 128 + 128],
                                    wo_sb[:, c * D + nh * 512:
                                          c * D + (nh + 1) * 512],
                                    start=(c == 0), stop=False)
                            nc.tensor.matmul(
                                po[:], ones[:],
                                bo_sb[:, nh * 512:(nh + 1) * 512],
                                start=False, stop=True)
                            outt = ostage.tile([128, 512], F32,
                                               name=f"ot{bh}_{ssub}_{nh}",
                                               tag="outt")
                            nc.vector.tensor_copy(outt[:], po[:])
                            nc.sync.dma_start(
                                out_slice[bh * ROWS + ssub * 128:
                                          bh * ROWS + ssub * 128 + 128,
                                          nh * 512:(nh + 1) * 512],
                                outt[:])

    nc.compile()
    return nc


def kernel(x, Wq, Wk, Wv, Wo, bo):
    if "nc" not in _CACHE:
        _CACHE["nc"] = _build()
    nc = _CACHE["nc"]

    x = np.ascontiguousarray(np.asarray(x, dtype=np.float32))
    Wq = np.asarray(Wq, dtype=np.float32)
    Wk = np.asarray(Wk, dtype=np.float32)
    Wv = np.asarray(Wv, dtype=np.float32)
    Wo = np.ascontiguousarray(np.asarray(Wo, dtype=np.float32))
    bo = np.asarray(bo, dtype=np.float32).reshape(1, D)

    in_maps = []
    for c in range(NC):
        b, hg = c // 4, c % 4
        cols = slice(hg * HW, (hg + 1) * HW)
        in_maps.append({
            "xin": np.ascontiguousarray(x[b]),
            "wq": np.ascontiguousarray(Wq[:, cols]),
            "wk": np.ascontiguousarray(Wk[:, cols]),
            "wv": np.ascontiguousarray(Wv[:, cols]),
            "wo": Wo,
            "bo": bo,
        })

    res = bass_utils.run_bass_kernel_spmd(nc, in_maps, core_ids=list(range(NC)))

    out = np.empty((2, S, D), dtype=np.float32)
    for j in range(NC):
        sl = res.results[j]["out_slice"]
        out[0, j * ROWS:(j + 1) * ROWS] = sl[:ROWS]
        out[1, j * ROWS:(j + 1) * ROWS] = sl[ROWS:]
    return out


# revision 15
# speedup vs baseline: 1.3768x; 1.1504x over previous
"""Multi-head causal attention (b=2, s=2048, d=1024, 16 heads) on 8 NeuronCores.

Sharding: head-tensor-parallel attention + Ulysses-style AllToAll.
  - Core c (c=0..7) owns batch c//4 and heads [4*(c%4), 4*(c%4)+4).
  - Each core: x^T via PE transpose, q^T/k^T (head-dim-major) + v (natural)
    projections, causal flash-style attention for its 4 heads -> ctx^T.
  - Two global 8-core AllToAlls (one per head pair, so the first overlaps
    with attention compute of the second pair) redistribute ctx^T so core j
    ends with all 1024 ctx features for rows [256*j, 256*(j+1)) of BOTH
    batches.
  - Sequence-parallel out-projection (full Wo, bias via K=1 ones matmul).
  - Host reassembles the 8 [512, 1024] row-slices. No all-reduce needed.

Matmuls run in float32r (tf32-like, 1 cycle/row at N>=256) with fp32 PSUM
accumulation; softmax runs unnormalized-exp (logits ~ N(0,1), no overflow)
with the denominator produced by an appended ones-column in the AV matmul.
Causal masking multiplies exp tiles by 4 precomputed diagonal masks on the
vector engine (keeps GpSimd out of the inner loop; PE stays HAM-warm).
"""

import sys

if "/opt/trn_rl_repo" not in sys.path:
    sys.path.insert(0, "/opt/trn_rl_repo")

import numpy as np

import concourse.bass as bass
import concourse.mybir as mybir
import concourse.bacc as bacc
import concourse.tile as tile
from concourse import bass_utils
from concourse.masks import make_identity

dt = mybir.dt
AF = mybir.ActivationFunctionType
F32 = dt.float32
F32R = dt.float32r

S = 2048          # sequence length
D = 1024          # model dim
NH = 16           # total heads
DH = 64           # head dim
NC = 8            # cores
HPC = NH // (NC // 2)   # heads per core = 4
HW = HPC * DH     # per-core head width = 256
QC = 512          # query chunk (free dim of S^T / AV matmuls)
NQC = S // QC     # 4
NST = S // 128    # 16 seq tiles
NKD = D // 128    # 8 contraction chunks over D
VW = HPC * (DH + 1)     # v tile width per seq-tile: 4 heads x (64 v + 1 ones)
ROWS = S // NC    # 256 rows per core per batch after A2A

_CACHE = {}


def _build():
    nc = bacc.Bacc("TRN2", debug=False, num_devices=NC)
    xin = nc.dram_tensor("xin", [S, D], F32R, kind="ExternalInput").ap()
    wq = nc.dram_tensor("wq", [D, HW], F32R, kind="ExternalInput").ap()
    wk = nc.dram_tensor("wk", [D, HW], F32R, kind="ExternalInput").ap()
    wv = nc.dram_tensor("wv", [D, HW], F32R, kind="ExternalInput").ap()
    wo = nc.dram_tensor("wo", [D, D], F32R, kind="ExternalInput").ap()
    bo = nc.dram_tensor("bo", [1, D], F32R, kind="ExternalInput").ap()
    out_slice = nc.dram_tensor(
        "out_slice", [2 * ROWS, D], F32, kind="ExternalOutput"
    ).ap()

    with tile.TileContext(nc) as tc:
        with (
            tc.tile_pool(name="constp", bufs=1) as constp,
            tc.tile_pool(name="wpool", bufs=1) as wpool,
            tc.tile_pool(name="dramp", bufs=1, space="DRAM") as dramp,
            tc.tile_pool(name="psum", bufs=1, space="PSUM") as psum,
        ):
            ident_f32 = constp.tile([128, 128], F32)
            make_identity(nc, ident_f32)
            ident = constp.tile([128, 128], F32R)
            nc.vector.tensor_copy(ident[:], ident_f32[:])
            ones_f32 = constp.tile([128, 128], F32)
            nc.vector.memset(ones_f32[:], 1.0)
            ones = constp.tile([1, 128], F32R)
            nc.vector.tensor_copy(ones[:], ones_f32[0:1, :])
            bo_sb = constp.tile([1, D], F32R)
            nc.sync.dma_start(bo_sb[:], bo[:])

            # 4 diagonal causal masks: mask[j][k, q] = (q - k - 128*j >= 0)
            masks = []
            mask_f32 = constp.tile([128, QC], F32)
            for j in range(4):
                nc.vector.memset(mask_f32[:], 1.0)
                nc.gpsimd.affine_select(
                    out=mask_f32[:], in_=mask_f32[:],
                    compare_op=mybir.AluOpType.is_ge,
                    fill=0.0, base=-128 * j,
                    pattern=[[1, QC]], channel_multiplier=-1)
                m = constp.tile([128, QC], F32R, name=f"mask{j}", tag=f"mask{j}")
                nc.vector.tensor_copy(m[:], mask_f32[:])
                masks.append(m)

            # weights: chunk kd of W* lives at columns [kd*w, (kd+1)*w)
            wq_sb = wpool.tile([128, NKD * HW], F32R)
            wk_sb = wpool.tile([128, NKD * HW], F32R)
            wv_sb = wpool.tile([128, NKD * HW], F32R)
            wo_sb = wpool.tile([128, NKD * D], F32R)
            for kd in range(NKD):
                r = slice(kd * 128, (kd + 1) * 128)
                nc.sync.dma_start(wq_sb[:, kd * HW:(kd + 1) * HW], wq[r, :])
                nc.sync.dma_start(wk_sb[:, kd * HW:(kd + 1) * HW], wk[r, :])
                nc.sync.dma_start(wv_sb[:, kd * HW:(kd + 1) * HW], wv[r, :])

            # bias broadcast to all 128 partitions, via K=1 ones matmul
            bias_sb = constp.tile([128, D], F32)
            for nh in range(2):
                pb = psum.tile([128, 512], F32, name=f"pb{nh}", tag="tp")
                nc.tensor.matmul(pb[:], ones[:],
                                 bo_sb[:, nh * 512:(nh + 1) * 512],
                                 start=True, stop=True)
                nc.scalar.copy(bias_sb[:, nh * 512:(nh + 1) * 512], pb[:])

            # A2A buffers, one per head, so early heads' collectives overlap
            # later heads' attention compute and only the last is exposed.
            a2a_in = [dramp.tile([NC, DH, ROWS], F32R, name=f"a2ai{i}",
                                 tag=f"a2ai{i}") for i in range(HPC)]
            a2a_out = [dramp.tile([NC, DH, ROWS], F32R, name=f"a2ao{i}",
                                  tag=f"a2ao{i}") for i in range(HPC)]

            with tc.tile_pool(name="qkvp", bufs=1) as qkvp:
                qT = qkvp.tile([128, 2 * S], F32R)   # head pair hp at cols hp*S
                kT = qkvp.tile([128, 2 * S], F32R)
                v_sb = qkvp.tile([128, NST * VW], F32R)
                # ones columns interleaved in v tiles: col st*VW + h*(DH+1) + DH
                v_view = v_sb[:].rearrange(
                    "p (st h c) -> p st h c", st=NST, h=HPC, c=DH + 1
                )
                nc.vector.tensor_copy(
                    v_view[:, :, :, DH:DH + 1],
                    ones_f32[:, 0:NST * HPC].rearrange(
                        "p (st h o) -> p st h o", st=NST, h=HPC, o=1))

                with tc.tile_pool(name="xtp", bufs=1) as xtp:
                    xT = xtp.tile([128, NKD * S], F32R)  # chunk kd at cols kd*S
                    xT_view = xT[:].rearrange("p (kd s) -> p kd s", kd=NKD)

                    with tc.tile_pool(name="xstage", bufs=3) as xsp:
                        for st in range(NST):
                            xst = xsp.tile([128, D], F32R, name=f"xst{st}",
                                           tag="xst")
                            nc.sync.dma_start(
                                xst[:], xin[st * 128:(st + 1) * 128, :])
                            for g in range(2):
                                pt = psum.tile([128, 512], F32R,
                                               name=f"tp{st}_{g}", tag="tp")
                                for kq in range(4):
                                    kd = g * 4 + kq
                                    nc.tensor.transpose(
                                        pt[:, kq * 128:(kq + 1) * 128],
                                        xst[:, kd * 128:(kd + 1) * 128],
                                        ident[:],
                                    )
                                dest = xT_view[
                                    :, g * 4:(g + 1) * 4,
                                    st * 128:(st + 1) * 128]
                                src = pt[:].rearrange("p (kq s) -> p kq s", kq=4)
                                nc.vector.tensor_copy(dest, src)

                    # q^T / k^T projections, head-pair packed (M=128)
                    for hp in range(2):
                        for qc in range(NQC):
                            pq = psum.tile([128, QC], F32,
                                           name=f"pq{hp}_{qc}", tag="pjq")
                            pk = psum.tile([128, QC], F32,
                                           name=f"pk{hp}_{qc}", tag="pjk")
                            for kd in range(NKD):
                                wcol = slice(kd * HW + hp * 128,
                                             kd * HW + (hp + 1) * 128)
                                xchunk = xT[:, kd * S + qc * QC:
                                            kd * S + (qc + 1) * QC]
                                nc.tensor.matmul(
                                    pq[:], wq_sb[:, wcol], xchunk,
                                    start=(kd == 0), stop=(kd == NKD - 1))
                                nc.tensor.matmul(
                                    pk[:], wk_sb[:, wcol], xchunk,
                                    start=(kd == 0), stop=(kd == NKD - 1))
                            dst = slice(hp * S + qc * QC, hp * S + (qc + 1) * QC)
                            nc.scalar.copy(qT[:, dst], pq[:])
                            nc.scalar.copy(kT[:, dst], pk[:])

                    # v natural [s, dh], 4 heads packed with ones columns
                    for st in range(NST):
                        pv = psum.tile([128, HW], F32, name=f"pv{st}",
                                       tag="pjq")
                        for kd in range(NKD):
                            nc.tensor.matmul(
                                pv[:],
                                xT_view[:, kd, st * 128:(st + 1) * 128],
                                wv_sb[:, kd * HW:(kd + 1) * HW],
                                start=(kd == 0), stop=(kd == NKD - 1))
                        nc.vector.tensor_copy(
                            v_view[:, st, :, 0:DH],
                            pv[:].rearrange("p (h d) -> p h d", h=HPC))

                # attention per (head, q-chunk), normalize software-pipelined
                # one iteration late so no engine queue stalls across iters
                with (
                    tc.tile_pool(name="pep", bufs=4) as pep,
                    tc.tile_pool(name="nrm", bufs=2) as nrm,
                ):
                    def normalize(h, qc, ctxp):
                        # ctx[d, q] /= denom[q] (denom = row DH of ctxp)
                        den = nrm.tile([1, QC], F32,
                                       name=f"den{h}_{qc}", tag="den")
                        nc.scalar.copy(den[:], ctxp[DH:DH + 1, :])
                        bc = nrm.tile([DH, QC], F32,
                                      name=f"bc{h}_{qc}", tag="bc")
                        nc.gpsimd.partition_broadcast(bc[:], den[:])
                        bcs = nrm.tile([DH, QC], F32,
                                       name=f"bcs{h}_{qc}", tag="bcs")
                        nc.vector.reciprocal_approx_fast(bcs[:], bc[:])
                        ctxn = nrm.tile([DH, QC], F32R,
                                        name=f"ctxn{h}_{qc}", tag="ctxn")
                        nc.vector.tensor_mul(ctxn[:], ctxp[0:DH, :], bcs[:])
                        for half in range(2):
                            nc.sync.dma_start(
                                a2a_in[h][2 * qc + half, :, :],
                                ctxn[:, half * ROWS:(half + 1) * ROWS])

                    pending = None  # (h, qc, ctxp) awaiting normalize
                    fire_cc = None  # head whose collective is due
                    for h in range(HPC):
                        hp, hl = h // 2, h % 2
                        prow = slice(hl * 64, (hl + 1) * 64)
                        for qc in range(NQC):
                            ctxp = psum.tile([DH + 1, QC], F32,
                                             name=f"ctx{h}_{qc}", tag="ctx")
                            nkt = 4 * (qc + 1)
                            for kt in range(nkt):
                                sp = psum.tile([128, QC], F32,
                                               name=f"sp{h}_{qc}_{kt}",
                                               tag="sp")
                                nc.tensor.matmul(
                                    sp[:],
                                    kT[prow, hp * S + kt * 128:
                                       hp * S + (kt + 1) * 128],
                                    qT[prow, hp * S + qc * QC:
                                       hp * S + (qc + 1) * QC],
                                    start=True, stop=True)
                                pe = pep.tile([128, QC], F32R,
                                              name=f"pe{h}_{qc}_{kt}",
                                              tag="pe")
                                nc.scalar.activation(
                                    pe[:], sp[:], AF.Exp, scale=0.125)
                                if kt >= 4 * qc:
                                    nc.vector.tensor_mul(
                                        pe[:], pe[:], masks[kt - 4 * qc][:])
                                nc.tensor.matmul(
                                    ctxp[:],
                                    v_sb[:, kt * VW + h * (DH + 1):
                                         kt * VW + (h + 1) * (DH + 1)],
                                    pe[:],
                                    start=(kt == 0), stop=(kt == nkt - 1))
                                if kt == 1 and pending is not None:
                                    normalize(*pending)
                                    pending = None
                                    if fire_cc is not None:
                                        nc.gpsimd.collective_compute(
                                            "AllToAll",
                                            mybir.AluOpType.bypass,
                                            replica_groups=[list(range(NC))],
                                            ins=[a2a_in[fire_cc].opt()],
                                            outs=[a2a_out[fire_cc].opt()])
                                        fire_cc = None
                            pending = (h, qc, ctxp)
                        fire_cc = h
                    normalize(*pending)
                    nc.gpsimd.collective_compute(
                        "AllToAll", mybir.AluOpType.bypass,
                        replica_groups=[list(range(NC))],
                        ins=[a2a_in[HPC - 1].opt()],
                        outs=[a2a_out[HPC - 1].opt()])

            # out-projection, sequence-parallel: 2*ROWS rows, full D columns
            for kd in range(NKD):
                nc.sync.dma_start(wo_sb[:, kd * D:(kd + 1) * D],
                                  wo[kd * 128:(kd + 1) * 128, :])
            with (
                tc.tile_pool(name="opool", bufs=1) as opool,
                tc.tile_pool(name="ostage", bufs=4) as ostage,
            ):
                # accumulate even chunks (heads 0/1, first two collectives)
                # before odd ones so out matmuls start as soon as data lands
                corder = [0, 2, 4, 6, 1, 3, 5, 7]
                for bh in range(2):
                    ctxo = opool.tile([128, NKD * ROWS], F32R,
                                      name=f"ctxo{bh}", tag=f"ctxo{bh}")
                    for c in range(NKD):
                        blk = bh * 4 + c // 2
                        ha = 2 * (c % 2)
                        nc.sync.dma_start(
                            ctxo[0:DH, c * ROWS:(c + 1) * ROWS],
                            a2a_out[ha][blk, :, :])
                        nc.sync.dma_start(
                            ctxo[DH:2 * DH, c * ROWS:(c + 1) * ROWS],
                            a2a_out[ha + 1][blk, :, :])
                    for ssub in range(ROWS // 128):
                        for nh in range(2):
                            po = psum.tile([128, 512], F32,
                                           name=f"po{bh}_{ssub}_{nh}",
                                           tag="pjk")
                            for ci, c in enumerate(corder):
                                nc.tensor.matmul(
                                    po[:],
                                    ctxo[:, c * ROWS + ssub * 128:
                                         c * ROWS + ssub * 128 + 128],
                                    wo_sb[:, c * D + nh * 512:
                                          c * D + (nh + 1) * 512],
                                    start=(ci == 0), stop=(ci == NKD - 1))
                            outt = ostage.tile([128, 512], F32,
                                               name=f"ot{bh}_{ssub}_{nh}",
                                               tag="outt")
                            nc.vector.tensor_add(
                                outt[:], po[:],
                                bias_sb[:, nh * 512:(nh + 1) * 512])
                            nc.sync.dma_start(
                                out_slice[bh * ROWS + ssub * 128:
                                          bh * ROWS + ssub * 128 + 128,
                                          nh * 512:(nh + 1) * 512],
                                outt[:])

    nc.compile()
    return nc


def kernel(x, Wq, Wk, Wv, Wo, bo):
    if "nc" not in _CACHE:
        _CACHE["nc"] = _build()
    nc = _CACHE["nc"]

    x = np.ascontiguousarray(np.asarray(x, dtype=np.float32))
    Wq = np.asarray(Wq, dtype=np.float32)
    Wk = np.asarray(Wk, dtype=np.float32)
    Wv = np.asarray(Wv, dtype=np.float32)
    Wo = np.ascontiguousarray(np.asarray(Wo, dtype=np.float32))
    bo = np.asarray(bo, dtype=np.float32).reshape(1, D)

    in_maps = []
    for c in range(NC):
        b, hg = c // 4, c % 4
        cols = slice(hg * HW, (hg + 1) * HW)
        in_maps.append({
            "xin": np.ascontiguousarray(x[b]),
            "wq": np.ascontiguousarray(Wq[:, cols]),
            "wk": np.ascontiguousarray(Wk[:, cols]),
            "wv": np.ascontiguousarray(Wv[:, cols]),
            "wo": Wo,
            "bo": bo,
        })

    res = bass_utils.run_bass_kernel_spmd(nc, in_maps, core_ids=list(range(NC)))

    out = np.empty((2, S, D), dtype=np.float32)
    for j in range(NC):
        sl = res.results[j]["out_slice"]
        out[0, j * ROWS:(j + 1) * ROWS] = sl[:ROWS]
        out[1, j * ROWS:(j + 1) * ROWS] = sl[ROWS:]
    return out


# revision 21
# speedup vs baseline: 1.4159x; 1.0284x over previous
"""Multi-head causal attention (b=2, s=2048, d=1024, 16 heads) on 8 NeuronCores.

Sharding: head-tensor-parallel attention + Ulysses-style AllToAll.
  - Core c (c=0..7) owns batch c//4 and heads [4*(c%4), 4*(c%4)+4).
  - Each core: x^T via PE transpose, q^T/k^T (head-dim-major) + v (natural)
    projections, causal flash-style attention for its 4 heads -> ctx^T.
  - Two global 8-core AllToAlls (one per head pair, so the first overlaps
    with attention compute of the second pair) redistribute ctx^T so core j
    ends with all 1024 ctx features for rows [256*j, 256*(j+1)) of BOTH
    batches.
  - Sequence-parallel out-projection (full Wo, bias via K=1 ones matmul).
  - Host reassembles the 8 [512, 1024] row-slices. No all-reduce needed.

Matmuls run in float32r (tf32-like, 1 cycle/row at N>=256) with fp32 PSUM
accumulation; softmax runs unnormalized-exp (logits ~ N(0,1), no overflow)
with the denominator produced by an appended ones-column in the AV matmul.
Causal masking multiplies exp tiles by 4 precomputed diagonal masks on the
vector engine (keeps GpSimd out of the inner loop; PE stays HAM-warm).
"""

import sys

if "/opt/trn_rl_repo" not in sys.path:
    sys.path.insert(0, "/opt/trn_rl_repo")

import numpy as np

import concourse.bass as bass
import concourse.mybir as mybir
import concourse.bacc as bacc
import concourse.tile as tile
from concourse import bass_utils
from concourse.masks import make_identity

dt = mybir.dt
AF = mybir.ActivationFunctionType
F32 = dt.float32
F32R = dt.float32r

S = 2048          # sequence length
D = 1024          # model dim
NH = 16           # total heads
DH = 64           # head dim
NC = 8            # cores
HPC = NH // (NC // 2)   # heads per core = 4
HW = HPC * DH     # per-core head width = 256
QC = 512          # query chunk (free dim of S^T / AV matmuls)
NQC = S // QC     # 4
NST = S // 128    # 16 seq tiles
NKD = D // 128    # 8 contraction chunks over D
VW = HPC * (DH + 1)     # v tile width per seq-tile: 4 heads x (64 v + 1 ones)
ROWS = S // NC    # 256 rows per core per batch after A2A

_CACHE = {}


def _build():
    nc = bacc.Bacc("TRN2", debug=False, num_devices=NC)
    xin = nc.dram_tensor("xin", [S, D], F32R, kind="ExternalInput").ap()
    wq = nc.dram_tensor("wq", [D, HW], F32R, kind="ExternalInput").ap()
    wk = nc.dram_tensor("wk", [D, HW], F32R, kind="ExternalInput").ap()
    wv = nc.dram_tensor("wv", [D, HW], F32R, kind="ExternalInput").ap()
    wo = nc.dram_tensor("wo", [D, D], F32R, kind="ExternalInput").ap()
    bo = nc.dram_tensor("bo", [1, D], F32R, kind="ExternalInput").ap()
    out_slice = nc.dram_tensor(
        "out_slice", [2 * ROWS, D], F32, kind="ExternalOutput"
    ).ap()

    with tile.TileContext(nc) as tc:
        with (
            tc.tile_pool(name="constp", bufs=1) as constp,
            tc.tile_pool(name="wpool", bufs=1) as wpool,
            tc.tile_pool(name="dramp", bufs=1, space="DRAM") as dramp,
            tc.tile_pool(name="psum", bufs=1, space="PSUM") as psum,
        ):
            ident_f32 = constp.tile([128, 128], F32)
            make_identity(nc, ident_f32)
            ident = constp.tile([128, 128], F32R)
            nc.vector.tensor_copy(ident[:], ident_f32[:])
            ones_f32 = constp.tile([128, 128], F32)
            nc.vector.memset(ones_f32[:], 1.0)
            ones = constp.tile([1, 128], F32R)
            nc.vector.tensor_copy(ones[:], ones_f32[0:1, :])
            bo_sb = constp.tile([1, D], F32R)
            nc.sync.dma_start(bo_sb[:], bo[:])

            # 4 diagonal causal masks: mask[j][k, q] = (q - k - 128*j >= 0)
            masks = []
            mask_f32 = constp.tile([128, QC], F32)
            for j in range(4):
                nc.vector.memset(mask_f32[:], 1.0)
                nc.gpsimd.affine_select(
                    out=mask_f32[:], in_=mask_f32[:],
                    compare_op=mybir.AluOpType.is_ge,
                    fill=0.0, base=-128 * j,
                    pattern=[[1, QC]], channel_multiplier=-1)
                m = constp.tile([128, QC], F32R, name=f"mask{j}", tag=f"mask{j}")
                nc.vector.tensor_copy(m[:], mask_f32[:])
                masks.append(m)

            # weights: chunk kd of W* lives at columns [kd*w, (kd+1)*w)
            wq_sb = wpool.tile([128, NKD * HW], F32R)
            wk_sb = wpool.tile([128, NKD * HW], F32R)
            wv_sb = wpool.tile([128, NKD * HW], F32R)
            wo_sb = wpool.tile([128, NKD * D], F32R)
            bias_sb = constp.tile([128, D], F32)

            # A2A buffers, one per head, so early heads' collectives overlap
            # later heads' attention compute and only the last is exposed.
            a2a_in = [dramp.tile([NC, DH, ROWS], F32R, name=f"a2ai{i}",
                                 tag=f"a2ai{i}") for i in range(HPC)]
            a2a_out = [dramp.tile([NC, DH, ROWS], F32R, name=f"a2ao{i}",
                                  tag=f"a2ao{i}") for i in range(HPC)]

            with tc.tile_pool(name="qkvp", bufs=1) as qkvp:
                qT = qkvp.tile([128, 2 * S], F32R)   # head pair hp at cols hp*S
                kT = qkvp.tile([128, 2 * S], F32R)
                v_sb = qkvp.tile([128, NST * VW], F32R)
                # ones columns interleaved in v tiles: col st*VW + h*(DH+1) + DH
                v_view = v_sb[:].rearrange(
                    "p (st h c) -> p st h c", st=NST, h=HPC, c=DH + 1
                )
                nc.vector.tensor_copy(
                    v_view[:, :, :, DH:DH + 1],
                    ones_f32[:, 0:NST * HPC].rearrange(
                        "p (st h o) -> p st h o", st=NST, h=HPC, o=1))

                with tc.tile_pool(name="xtp", bufs=1) as xtp:
                    xT = xtp.tile([128, NKD * S], F32R)  # chunk kd at cols kd*S
                    xT_view = xT[:].rearrange("p (kd s) -> p kd s", kd=NKD)

                    with tc.tile_pool(name="xstage", bufs=3) as xsp:
                        for st in range(NST):
                            xst = xsp.tile([128, D], F32R, name=f"xst{st}",
                                           tag="xst")
                            nc.sync.dma_start(
                                xst[:], xin[st * 128:(st + 1) * 128, :])
                            for g in range(2):
                                pt = psum.tile([128, 512], F32R,
                                               name=f"tp{st}_{g}", tag="tp")
                                for kq in range(4):
                                    kd = g * 4 + kq
                                    nc.tensor.transpose(
                                        pt[:, kq * 128:(kq + 1) * 128],
                                        xst[:, kd * 128:(kd + 1) * 128],
                                        ident[:],
                                    )
                                dest = xT_view[
                                    :, g * 4:(g + 1) * 4,
                                    st * 128:(st + 1) * 128]
                                src = pt[:].rearrange("p (kq s) -> p kq s", kq=4)
                                nc.vector.tensor_copy(dest, src)

                    # weight loads AFTER x in the sync queue (x gates the
                    # transposes; weights aren't needed until projections).
                    # One 3D-AP DMA per weight matrix.
                    for w_sb, w_dram in ((wq_sb, wq), (wk_sb, wk),
                                         (wv_sb, wv)):
                        nc.sync.dma_start(
                            w_sb[:].rearrange("p (kd w) -> p kd w", kd=NKD),
                            w_dram.rearrange("(kd p) w -> p kd w", p=128))

                    # bias broadcast to all 128 partitions via K=1 ones matmul
                    for nh in range(2):
                        pb = psum.tile([128, 512], F32, name=f"pb{nh}",
                                       tag="tp")
                        nc.tensor.matmul(pb[:], ones[:],
                                         bo_sb[:, nh * 512:(nh + 1) * 512],
                                         start=True, stop=True)
                        nc.scalar.copy(bias_sb[:, nh * 512:(nh + 1) * 512],
                                       pb[:])

                    # q^T / k^T projections, head-pair packed (M=128)
                    for hp in range(2):
                        for qc in range(NQC):
                            pq = psum.tile([128, QC], F32,
                                           name=f"pq{hp}_{qc}", tag="pjq")
                            pk = psum.tile([128, QC], F32,
                                           name=f"pk{hp}_{qc}", tag="pjk")
                            for kd in range(NKD):
                                wcol = slice(kd * HW + hp * 128,
                                             kd * HW + (hp + 1) * 128)
                                xchunk = xT[:, kd * S + qc * QC:
                                            kd * S + (qc + 1) * QC]
                                nc.tensor.matmul(
                                    pq[:], wq_sb[:, wcol], xchunk,
                                    start=(kd == 0), stop=(kd == NKD - 1))
                                nc.tensor.matmul(
                                    pk[:], wk_sb[:, wcol], xchunk,
                                    start=(kd == 0), stop=(kd == NKD - 1))
                            dst = slice(hp * S + qc * QC, hp * S + (qc + 1) * QC)
                            nc.scalar.copy(qT[:, dst], pq[:])
                            nc.scalar.copy(kT[:, dst], pk[:])

                    # v natural [s, dh], 4 heads packed with ones columns
                    for st in range(NST):
                        pv = psum.tile([128, HW], F32, name=f"pv{st}",
                                       tag="pjq")
                        for kd in range(NKD):
                            nc.tensor.matmul(
                                pv[:],
                                xT_view[:, kd, st * 128:(st + 1) * 128],
                                wv_sb[:, kd * HW:(kd + 1) * HW],
                                start=(kd == 0), stop=(kd == NKD - 1))
                        nc.vector.tensor_copy(
                            v_view[:, st, :, 0:DH],
                            pv[:].rearrange("p (h d) -> p h d", h=HPC))

                # attention per (head, q-chunk), normalize software-pipelined
                # one iteration late so no engine queue stalls across iters
                with (
                    tc.tile_pool(name="pep", bufs=4) as pep,
                    tc.tile_pool(name="nrm", bufs=2) as nrm,
                ):
                    def normalize(h, qc, ctxp):
                        # ctx[d, q] /= denom[q] (denom = row DH of ctxp)
                        den = nrm.tile([1, QC], F32R,
                                       name=f"den{h}_{qc}", tag="den")
                        nc.scalar.copy(den[:], ctxp[DH:DH + 1, :])
                        bc = psum.tile([DH, QC], F32,
                                       name=f"bc{h}_{qc}", tag="tp")
                        nc.tensor.matmul(bc[:], ones[:, 0:DH], den[:],
                                         start=True, stop=True)
                        bcs = nrm.tile([DH, QC], F32,
                                       name=f"bcs{h}_{qc}", tag="bcs")
                        nc.vector.reciprocal_approx_fast(bcs[:], bc[:])
                        ctxn = nrm.tile([DH, QC], F32R,
                                        name=f"ctxn{h}_{qc}", tag="ctxn")
                        nc.vector.tensor_mul(ctxn[:], ctxp[0:DH, :], bcs[:])
                        nc.sync.dma_start(
                            a2a_in[h][2 * qc:2 * qc + 2, :, :].rearrange(
                                "j p r -> p j r"),
                            ctxn[:])

                    pending = None  # (h, qc, ctxp) awaiting normalize
                    fire_cc = None  # head whose collective is due
                    for h in range(HPC):
                        if h == 1:
                            # Wo load on the ACT-triggered HWDGE queue: fires
                            # mid-attention, off the congested sync queue.
                            nc.scalar.dma_start(
                                wo_sb[:].rearrange("p (kd w) -> p kd w",
                                                   kd=NKD),
                                wo.rearrange("(kd p) w -> p kd w", p=128))
                        hp, hl = h // 2, h % 2
                        prow = slice(hl * 64, (hl + 1) * 64)
                        for qc in range(NQC):
                            ctxp = psum.tile([DH + 1, QC], F32,
                                             name=f"ctx{h}_{qc}", tag="ctx")
                            nkt = 4 * (qc + 1)
                            for kt in range(nkt):
                                sp = psum.tile([128, QC], F32,
                                               name=f"sp{h}_{qc}_{kt}",
                                               tag="sp")
                                nc.tensor.matmul(
                                    sp[:],
                                    kT[prow, hp * S + kt * 128:
                                       hp * S + (kt + 1) * 128],
                                    qT[prow, hp * S + qc * QC:
                                       hp * S + (qc + 1) * QC],
                                    start=True, stop=True)
                                pe = pep.tile([128, QC], F32R,
                                              name=f"pe{h}_{qc}_{kt}",
                                              tag="pe")
                                nc.scalar.activation(
                                    pe[:], sp[:], AF.Exp, scale=0.125)
                                if kt >= 4 * qc:
                                    nc.vector.tensor_mul(
                                        pe[:], pe[:], masks[kt - 4 * qc][:])
                                nc.tensor.matmul(
                                    ctxp[:],
                                    v_sb[:, kt * VW + h * (DH + 1):
                                         kt * VW + (h + 1) * (DH + 1)],
                                    pe[:],
                                    start=(kt == 0), stop=(kt == nkt - 1))
                                if kt == 1 and pending is not None:
                                    normalize(*pending)
                                    pending = None
                                    if fire_cc is not None:
                                        nc.gpsimd.collective_compute(
                                            "AllToAll",
                                            mybir.AluOpType.bypass,
                                            replica_groups=[list(range(NC))],
                                            ins=[a2a_in[fire_cc].opt()],
                                            outs=[a2a_out[fire_cc].opt()])
                                        fire_cc = None
                            pending = (h, qc, ctxp)
                        fire_cc = h
                    normalize(*pending)
                    nc.gpsimd.collective_compute(
                        "AllToAll", mybir.AluOpType.bypass,
                        replica_groups=[list(range(NC))],
                        ins=[a2a_in[HPC - 1].opt()],
                        outs=[a2a_out[HPC - 1].opt()])

            # out-projection, sequence-parallel: 2*ROWS rows, full D columns
            with (
                tc.tile_pool(name="opool", bufs=1) as opool,
                tc.tile_pool(name="ostage", bufs=4) as ostage,
            ):
                # accumulate even chunks (heads 0/1, first two collectives)
                # before odd ones so out matmuls start as soon as data lands
                corder = [0, 2, 4, 6, 1, 3, 5, 7]
                for bh in range(2):
                    ctxo = opool.tile([128, NKD * ROWS], F32R,
                                      name=f"ctxo{bh}", tag=f"ctxo{bh}")
                    for c in range(NKD):
                        blk = bh * 4 + c // 2
                        ha = 2 * (c % 2)
                        nc.sync.dma_start(
                            ctxo[0:DH, c * ROWS:(c + 1) * ROWS],
                            a2a_out[ha][blk, :, :])
                        nc.sync.dma_start(
                            ctxo[DH:2 * DH, c * ROWS:(c + 1) * ROWS],
                            a2a_out[ha + 1][blk, :, :])
                    for ssub in range(ROWS // 128):
                        for nh in range(2):
                            po = psum.tile([128, 512], F32,
                                           name=f"po{bh}_{ssub}_{nh}",
                                           tag="pjk")
                            for ci, c in enumerate(corder):
                                nc.tensor.matmul(
                                    po[:],
                                    ctxo[:, c * ROWS + ssub * 128:
                                         c * ROWS + ssub * 128 + 128],
                                    wo_sb[:, c * D + nh * 512:
                                          c * D + (nh + 1) * 512],
                                    start=(ci == 0), stop=(ci == NKD - 1))
                            outt = ostage.tile([128, 512], F32,
                                               name=f"ot{bh}_{ssub}_{nh}",
                                               tag="outt")
                            nc.vector.tensor_add(
                                outt[:], po[:],
                                bias_sb[:, nh * 512:(nh + 1) * 512])
                            nc.sync.dma_start(
                                out_slice[bh * ROWS + ssub * 128:
                                          bh * ROWS + ssub * 128 + 128,
                                          nh * 512:(nh + 1) * 512],
                                outt[:])

    nc.compile()
    return nc


def kernel(x, Wq, Wk, Wv, Wo, bo):
    if "nc" not in _CACHE:
        _CACHE["nc"] = _build()
    nc = _CACHE["nc"]

    x = np.ascontiguousarray(np.asarray(x, dtype=np.float32))
    Wq = np.asarray(Wq, dtype=np.float32)
    Wk = np.asarray(Wk, dtype=np.float32)
    Wv = np.asarray(Wv, dtype=np.float32)
    Wo = np.ascontiguousarray(np.asarray(Wo, dtype=np.float32))
    bo = np.asarray(bo, dtype=np.float32).reshape(1, D)

    in_maps = []
    for c in range(NC):
        b, hg = c // 4, c % 4
        cols = slice(hg * HW, (hg + 1) * HW)
        in_maps.append({
            "xin": np.ascontiguousarray(x[b]),
            "wq": np.ascontiguousarray(Wq[:, cols]),
            "wk": np.ascontiguousarray(Wk[:, cols]),
            "wv": np.ascontiguousarray(Wv[:, cols]),
            "wo": Wo,
            "bo": bo,
        })

    res = bass_utils.run_bass_kernel_spmd(nc, in_maps, core_ids=list(range(NC)))

    out = np.empty((2, S, D), dtype=np.float32)
    for j in range(NC):
        sl = res.results[j]["out_slice"]
        out[0, j * ROWS:(j + 1) * ROWS] = sl[:ROWS]
        out[1, j * ROWS:(j + 1) * ROWS] = sl[ROWS:]
    return out


# revision 25
# speedup vs baseline: 1.9328x; 1.3651x over previous
"""Multi-head causal attention (b=2, s=2048, d=1024, 16 heads) on 8 NeuronCores.

Sharding: head-tensor-parallel attention + Ulysses-style AllToAll.
  - Core c (c=0..7) owns batch c//4 and heads [4*(c%4), 4*(c%4)+4).
  - Each core: x^T via PE transpose, q^T/k^T (head-dim-major) + v (natural)
    projections, causal flash-style attention for its 4 heads -> ctx^T.
  - Two global 8-core AllToAlls (one per head pair, so the first overlaps
    with attention compute of the second pair) redistribute ctx^T so core j
    ends with all 1024 ctx features for rows [256*j, 256*(j+1)) of BOTH
    batches.
  - Sequence-parallel out-projection (full Wo, bias via K=1 ones matmul).
  - Host reassembles the 8 [512, 1024] row-slices. No all-reduce needed.

Matmuls run in float32r (tf32-like, 1 cycle/row at N>=256) with fp32 PSUM
accumulation; softmax runs unnormalized-exp (logits ~ N(0,1), no overflow)
with the denominator produced by an appended ones-column in the AV matmul.
Causal masking multiplies exp tiles by 4 precomputed diagonal masks on the
vector engine (keeps GpSimd out of the inner loop; PE stays HAM-warm).
"""

import sys

if "/opt/trn_rl_repo" not in sys.path:
    sys.path.insert(0, "/opt/trn_rl_repo")

import numpy as np

import concourse.bass as bass
import concourse.mybir as mybir
import concourse.bacc as bacc
import concourse.tile as tile
from concourse import bass_utils
from concourse.masks import make_identity

dt = mybir.dt
AF = mybir.ActivationFunctionType
F32 = dt.float32
F32R = dt.float32r

S = 2048          # sequence length
D = 1024          # model dim
NH = 16           # total heads
DH = 64           # head dim
NC = 8            # cores
HPC = NH // (NC // 2)   # heads per core = 4
HW = HPC * DH     # per-core head width = 256
QC = 512          # query chunk (free dim of S^T / AV matmuls)
NQC = S // QC     # 4
NST = S // 128    # 16 seq tiles
NKD = D // 128    # 8 contraction chunks over D
VW = HPC * (DH + 1)     # v tile width per seq-tile: 4 heads x (64 v + 1 ones)
ROWS = S // NC    # 256 rows per core per batch after A2A

_CACHE = {}


def _build():
    nc = bacc.Bacc("TRN2", debug=False, num_devices=NC)
    xin = nc.dram_tensor("xin", [S, D], F32R, kind="ExternalInput").ap()
    wq = nc.dram_tensor("wq", [D, HW], F32R, kind="ExternalInput").ap()
    wk = nc.dram_tensor("wk", [D, HW], F32R, kind="ExternalInput").ap()
    wv = nc.dram_tensor("wv", [D, HW], F32R, kind="ExternalInput").ap()
    wo = nc.dram_tensor("wo", [D, D], F32R, kind="ExternalInput").ap()
    bo = nc.dram_tensor("bo", [1, D], F32R, kind="ExternalInput").ap()
    out_slice = nc.dram_tensor(
        "out_slice", [2 * ROWS, D], F32, kind="ExternalOutput"
    ).ap()

    with tile.TileContext(nc) as tc:
        with (
            tc.tile_pool(name="constp", bufs=1) as constp,
            tc.tile_pool(name="wpool", bufs=1) as wpool,
            tc.tile_pool(name="dramp", bufs=1, space="DRAM") as dramp,
            tc.tile_pool(name="psp", bufs=4, space="PSUM") as psp,
            tc.tile_pool(name="pjp", bufs=2, space="PSUM") as pjp,
            tc.tile_pool(name="ctxpool", bufs=2, space="PSUM") as ctxpool,
        ):
            ident_f32 = constp.tile([128, 128], F32)
            make_identity(nc, ident_f32)
            ident = constp.tile([128, 128], F32R)
            nc.vector.tensor_copy(ident[:], ident_f32[:])
            ones_f32 = constp.tile([128, 128], F32)
            nc.vector.memset(ones_f32[:], 1.0)
            ones = constp.tile([1, 128], F32R)
            nc.vector.tensor_copy(ones[:], ones_f32[0:1, :])
            bo_sb = constp.tile([1, D], F32R)
            nc.sync.dma_start(bo_sb[:], bo[:])

            # 4 diagonal causal masks: mask[j][k, q] = (q - k - 128*j >= 0)
            masks = []
            mask_f32 = constp.tile([128, QC], F32)
            for j in range(4):
                nc.vector.memset(mask_f32[:], 1.0)
                nc.gpsimd.affine_select(
                    out=mask_f32[:], in_=mask_f32[:],
                    compare_op=mybir.AluOpType.is_ge,
                    fill=0.0, base=-128 * j,
                    pattern=[[1, QC]], channel_multiplier=-1)
                m = constp.tile([128, QC], F32R, name=f"mask{j}", tag=f"mask{j}")
                nc.vector.tensor_copy(m[:], mask_f32[:])
                masks.append(m)

            # weights: chunk kd of W* lives at columns [kd*w, (kd+1)*w)
            wq_sb = wpool.tile([128, NKD * HW], F32R)
            wk_sb = wpool.tile([128, NKD * HW], F32R)
            wv_sb = wpool.tile([128, NKD * HW], F32R)
            wo_sb = wpool.tile([128, NKD * D], F32R)
            bias_sb = constp.tile([128, D], F32)

            # A2A buffers, one per head, so early heads' collectives overlap
            # later heads' attention compute and only the last is exposed.
            a2a_in = [dramp.tile([NC, DH, ROWS], F32R, name=f"a2ai{i}",
                                 tag=f"a2ai{i}") for i in range(HPC)]
            a2a_out = [dramp.tile([NC, DH, ROWS], F32R, name=f"a2ao{i}",
                                  tag=f"a2ao{i}") for i in range(HPC)]

            with tc.tile_pool(name="qkvp", bufs=1) as qkvp:
                qT = qkvp.tile([128, 2 * S], F32R)   # head pair hp at cols hp*S
                kT = qkvp.tile([128, 2 * S], F32R)
                v_sb = qkvp.tile([128, NST * VW], F32R)
                # ones columns interleaved in v tiles: col st*VW + h*(DH+1) + DH
                v_view = v_sb[:].rearrange(
                    "p (st h c) -> p st h c", st=NST, h=HPC, c=DH + 1
                )
                nc.vector.tensor_copy(
                    v_view[:, :, :, DH:DH + 1],
                    ones_f32[:, 0:NST * HPC].rearrange(
                        "p (st h o) -> p st h o", st=NST, h=HPC, o=1))

                with tc.tile_pool(name="xtp", bufs=1) as xtp:
                    xT = xtp.tile([128, NKD * S], F32R)  # chunk kd at cols kd*S
                    xT_view = xT[:].rearrange("p (kd s) -> p kd s", kd=NKD)

                    with tc.tile_pool(name="xstage", bufs=3) as xsp:
                        for st in range(NST):
                            xst = xsp.tile([128, D], F32R, name=f"xst{st}",
                                           tag="xst")
                            nc.sync.dma_start(
                                xst[:], xin[st * 128:(st + 1) * 128, :])
                            for g in range(2):
                                pt = psp.tile([128, 512], F32R,
                                               name=f"tp{st}_{g}", tag="sp")
                                for kq in range(4):
                                    kd = g * 4 + kq
                                    nc.tensor.transpose(
                                        pt[:, kq * 128:(kq + 1) * 128],
                                        xst[:, kd * 128:(kd + 1) * 128],
                                        ident[:],
                                    )
                                dest = xT_view[
                                    :, g * 4:(g + 1) * 4,
                                    st * 128:(st + 1) * 128]
                                src = pt[:].rearrange("p (kq s) -> p kq s", kq=4)
                                nc.vector.tensor_copy(dest, src)

                    # weight loads AFTER x in the sync queue (x gates the
                    # transposes; weights aren't needed until projections).
                    # One 3D-AP DMA per weight matrix.
                    for w_sb, w_dram in ((wq_sb, wq), (wk_sb, wk),
                                         (wv_sb, wv)):
                        nc.sync.dma_start(
                            w_sb[:].rearrange("p (kd w) -> p kd w", kd=NKD),
                            w_dram.rearrange("(kd p) w -> p kd w", p=128))

                    # bias broadcast to all 128 partitions via K=1 ones matmul
                    for nh in range(2):
                        pb = psp.tile([128, 512], F32, name=f"pb{nh}",
                                       tag="sp")
                        nc.tensor.matmul(pb[:], ones[:],
                                         bo_sb[:, nh * 512:(nh + 1) * 512],
                                         start=True, stop=True)
                        nc.scalar.copy(bias_sb[:, nh * 512:(nh + 1) * 512],
                                       pb[:])

                    # q^T / k^T projections, head-pair packed (M=128)
                    for hp in range(2):
                        for qc in range(NQC):
                            pq = pjp.tile([128, QC], F32,
                                           name=f"pq{hp}_{qc}", tag="pj")
                            pk = pjp.tile([128, QC], F32,
                                           name=f"pk{hp}_{qc}", tag="pj")
                            for kd in range(NKD):
                                wcol = slice(kd * HW + hp * 128,
                                             kd * HW + (hp + 1) * 128)
                                xchunk = xT[:, kd * S + qc * QC:
                                            kd * S + (qc + 1) * QC]
                                nc.tensor.matmul(
                                    pq[:], wq_sb[:, wcol], xchunk,
                                    start=(kd == 0), stop=(kd == NKD - 1))
                                nc.tensor.matmul(
                                    pk[:], wk_sb[:, wcol], xchunk,
                                    start=(kd == 0), stop=(kd == NKD - 1))
                            dst = slice(hp * S + qc * QC, hp * S + (qc + 1) * QC)
                            nc.scalar.copy(qT[:, dst], pq[:])
                            nc.scalar.copy(kT[:, dst], pk[:])

                    # v natural [s, dh], 4 heads packed with ones columns
                    for st in range(NST):
                        pv = pjp.tile([128, HW], F32, name=f"pv{st}",
                                       tag="pj")
                        for kd in range(NKD):
                            nc.tensor.matmul(
                                pv[:],
                                xT_view[:, kd, st * 128:(st + 1) * 128],
                                wv_sb[:, kd * HW:(kd + 1) * HW],
                                start=(kd == 0), stop=(kd == NKD - 1))
                        nc.vector.tensor_copy(
                            v_view[:, st, :, 0:DH],
                            pv[:].rearrange("p (h d) -> p h d", h=HPC))

                # attention per (head, q-chunk), normalize software-pipelined
                # one iteration late so no engine queue stalls across iters
                with (
                    tc.tile_pool(name="pep", bufs=4) as pep,
                    tc.tile_pool(name="nrm", bufs=2) as nrm,
                ):
                    def normalize(h, qc, ctxp):
                        # ctx[d, q] /= denom[q] (denom = row DH of ctxp)
                        den = nrm.tile([1, QC], F32R,
                                       name=f"den{h}_{qc}", tag="den")
                        nc.scalar.copy(den[:], ctxp[DH:DH + 1, :])
                        bc = psp.tile([DH, QC], F32,
                                       name=f"bc{h}_{qc}", tag="sp")
                        nc.tensor.matmul(bc[:], ones[:, 0:DH], den[:],
                                         start=True, stop=True)
                        bcs = nrm.tile([DH, QC], F32,
                                       name=f"bcs{h}_{qc}", tag="bcs")
                        nc.vector.reciprocal_approx_fast(bcs[:], bc[:])
                        ctxn = nrm.tile([DH, QC], F32R,
                                        name=f"ctxn{h}_{qc}", tag="ctxn")
                        nc.vector.tensor_mul(ctxn[:], ctxp[0:DH, :], bcs[:])
                        nc.sync.dma_start(
                            a2a_in[h][2 * qc:2 * qc + 2, :, :].rearrange(
                                "j p r -> p j r"),
                            ctxn[:])

                    pending = None  # (h, qc, ctxp) awaiting normalize
                    fire_cc = None  # head whose collective is due
                    for h in range(HPC):
                        if h == 1:
                            # Wo load on the ACT-triggered HWDGE queue: fires
                            # mid-attention, off the congested sync queue.
                            nc.scalar.dma_start(
                                wo_sb[:].rearrange("p (kd w) -> p kd w",
                                                   kd=NKD),
                                wo.rearrange("(kd p) w -> p kd w", p=128))
                        hp, hl = h // 2, h % 2
                        prow = slice(hl * 64, (hl + 1) * 64)
                        for qc in range(NQC):
                            ctxp = ctxpool.tile([DH + 1, QC], F32,
                                             name=f"ctx{h}_{qc}", tag="ctx")
                            nkt = 4 * (qc + 1)
                            pes = {}

                            def emit_s(kt, h=h, qc=qc, hp=hp, prow=prow,
                                       pes=pes):
                                # S^T tile + exp + causal mask; AV emitted
                                # LOOK tiles behind so PE never queue-stalls
                                sp = psp.tile([128, QC], F32,
                                               name=f"sp{h}_{qc}_{kt}",
                                               tag="sp")
                                nc.tensor.matmul(
                                    sp[:],
                                    kT[prow, hp * S + kt * 128:
                                       hp * S + (kt + 1) * 128],
                                    qT[prow, hp * S + qc * QC:
                                       hp * S + (qc + 1) * QC],
                                    start=True, stop=True)
                                pe = pep.tile([128, QC], F32R,
                                              name=f"pe{h}_{qc}_{kt}",
                                              tag="pe")
                                nc.scalar.activation(
                                    pe[:], sp[:], AF.Exp, scale=0.125)
                                if kt >= 4 * qc:
                                    nc.vector.tensor_mul(
                                        pe[:], pe[:], masks[kt - 4 * qc][:])
                                pes[kt] = pe

                            LOOK = 3
                            for kt in range(min(LOOK, nkt)):
                                emit_s(kt)
                            for kt in range(nkt):
                                if kt + LOOK < nkt:
                                    emit_s(kt + LOOK)
                                nc.tensor.matmul(
                                    ctxp[:],
                                    v_sb[:, kt * VW + h * (DH + 1):
                                         kt * VW + (h + 1) * (DH + 1)],
                                    pes.pop(kt)[:],
                                    start=(kt == 0), stop=(kt == nkt - 1))
                                if kt == 1 and pending is not None:
                                    normalize(*pending)
                                    pending = None
                                    if fire_cc is not None:
                                        nc.gpsimd.collective_compute(
                                            "AllToAll",
                                            mybir.AluOpType.bypass,
                                            replica_groups=[list(range(NC))],
                                            ins=[a2a_in[fire_cc].opt()],
                                            outs=[a2a_out[fire_cc].opt()])
                                        fire_cc = None
                            pending = (h, qc, ctxp)
                        fire_cc = h
                    normalize(*pending)
                    nc.gpsimd.collective_compute(
                        "AllToAll", mybir.AluOpType.bypass,
                        replica_groups=[list(range(NC))],
                        ins=[a2a_in[HPC - 1].opt()],
                        outs=[a2a_out[HPC - 1].opt()])

            # out-projection, sequence-parallel: 2*ROWS rows, full D columns
            with (
                tc.tile_pool(name="opool", bufs=1) as opool,
                tc.tile_pool(name="ostage", bufs=4) as ostage,
            ):
                # accumulate even chunks (heads 0/1, first two collectives)
                # before odd ones so out matmuls start as soon as data lands
                corder = [0, 2, 4, 6, 1, 3, 5, 7]
                for bh in range(2):
                    ctxo = opool.tile([128, NKD * ROWS], F32R,
                                      name=f"ctxo{bh}", tag=f"ctxo{bh}")
                    for c in range(NKD):
                        blk = bh * 4 + c // 2
                        ha = 2 * (c % 2)
                        nc.sync.dma_start(
                            ctxo[0:DH, c * ROWS:(c + 1) * ROWS],
                            a2a_out[ha][blk, :, :])
                        nc.sync.dma_start(
                            ctxo[DH:2 * DH, c * ROWS:(c + 1) * ROWS],
                            a2a_out[ha + 1][blk, :, :])
                    for ssub in range(ROWS // 128):
                        for nh in range(2):
                            po = pjp.tile([128, 512], F32,
                                           name=f"po{bh}_{ssub}_{nh}",
                                           tag="pj")
                            for ci, c in enumerate(corder):
                                nc.tensor.matmul(
                                    po[:],
                                    ctxo[:, c * ROWS + ssub * 128:
                                         c * ROWS + ssub * 128 + 128],
                                    wo_sb[:, c * D + nh * 512:
                                          c * D + (nh + 1) * 512],
                                    start=(ci == 0), stop=(ci == NKD - 1))
                            outt = ostage.tile([128, 512], F32,
                                               name=f"ot{bh}_{ssub}_{nh}",
                                               tag="outt")
                            nc.vector.tensor_add(
                                outt[:], po[:],
                                bias_sb[:, nh * 512:(nh + 1) * 512])
                            nc.sync.dma_start(
                                out_slice[bh * ROWS + ssub * 128:
                                          bh * ROWS + ssub * 128 + 128,
                                          nh * 512:(nh + 1) * 512],
                                outt[:])

    nc.compile()
    return nc


def kernel(x, Wq, Wk, Wv, Wo, bo):
    if "nc" not in _CACHE:
        _CACHE["nc"] = _build()
    nc = _CACHE["nc"]

    x = np.ascontiguousarray(np.asarray(x, dtype=np.float32))
    Wq = np.asarray(Wq, dtype=np.float32)
    Wk = np.asarray(Wk, dtype=np.float32)
    Wv = np.asarray(Wv, dtype=np.float32)
    Wo = np.ascontiguousarray(np.asarray(Wo, dtype=np.float32))
    bo = np.asarray(bo, dtype=np.float32).reshape(1, D)

    in_maps = []
    for c in range(NC):
        b, hg = c // 4, c % 4
        cols = slice(hg * HW, (hg + 1) * HW)
        in_maps.append({
            "xin": np.ascontiguousarray(x[b]),
            "wq": np.ascontiguousarray(Wq[:, cols]),
            "wk": np.ascontiguousarray(Wk[:, cols]),
            "wv": np.ascontiguousarray(Wv[:, cols]),
            "wo": Wo,
            "bo": bo,
        })

    res = bass_utils.run_bass_kernel_spmd(nc, in_maps, core_ids=list(range(NC)))

    out = np.empty((2, S, D), dtype=np.float32)
    for j in range(NC):
        sl = res.results[j]["out_slice"]
        out[0, j * ROWS:(j + 1) * ROWS] = sl[:ROWS]
        out[1, j * ROWS:(j + 1) * ROWS] = sl[ROWS:]
    return out
